# revision 34
# baseline (speedup 1.0000x reference)
"""PointNet Feature Propagation kernel for Trainium2 (8 NeuronCores, SPMD).

Data-parallel over N: each core owns 2048 of 16384 query points; xyz2/points2
and the conv/BN params are replicated.

Pipeline per core, streamed tile-by-tile (16 tiles of 128 points):
  1. Distance matrix as ONE K=24 bf16 matmul group per tile: CPU pre-encodes
     xyz as hi/mid/lo bf16 triples and stacks the cross terms PLUS the
     -|x1|^2 / -|x2|^2 norm rows along K, so psum = -d^2 exactly to ~1e-7.
     The dist matmul whose bank holds the rotating transpose region is
     emitted LAST so the other three never wait on the interp-copy release.
  2. DVE is the roofline engine and sets the ~5.5us/tile cadence: max8 +
     find_index8 (the two unavoidable full-S passes) plus tiny d6 ops and
     the weight reciprocal.  d6 is computed ON the DVE right after max8 so
     every reader of the bufs=1 mx tile is a DVE op: max8(t+1)'s WAR on mx
     then follows pure DVE program order, which stops the Tile scheduler
     from statically hoisting max8(t+1) ahead of fi8(t) (that pairing
     couples one tile's gather->transpose->copy chain serially with two
     full DVE scans and was worth ~25us).
  3. Neighbor rows: 3 per-k indirect row-gathers per tile from a bf16 DRAM
     table (HW SWDGE only honors one offset per partition - batched
     multi-offset gathers return garbage on HW even though CoreSim accepts
     them).
  4. dg_k = diag(w_k) built on ACT concurrently with the gathers; the PE
     then does weighted-sum + transpose-to-feature-major in one 3-matmul
     accumulation group per 128-col half, into rotating 2x128-col psum
     slots carved from the opposite-parity distance buffer, ACT-copied out.
  5. The two 1x1-conv layers (BN folded on CPU, ReLU+bias fused into one
     ACT per chunk, all psum dests bank-aligned) run as three chunks:
     tiles 0-7 start right after fi8(14) frees psum buffer 0 (overlapping
     tile 15's selection); tiles 8-11 and 12-15 follow.
"""
import ml_dtypes
import numpy as np

import concourse.bacc as bacc
import concourse.bass as bass
import concourse.mybir as mybir
from concourse import bass_utils
from concourse.masks import make_identity
from concourse.tile import TileContext

f32 = mybir.dt.float32
bf16 = mybir.dt.bfloat16
u32 = mybir.dt.uint32

NCORES = 8
N = 16384
NLOC = N // NCORES          # 2048 points per core
S = 2048                    # reference points (replicated)
D1 = 128                    # points1 channels
D2 = 256                    # points2 channels
M0 = 256                    # mlp hidden
M1 = 128                    # mlp out
NT = NLOC // 128            # 16 point-tiles per core
K = 24                      # augmented contraction rows
BN_EPS = 1e-5
GATHER_BATCHED = False      # HW SWDGE only supports 1 offset/partition

# MLP chunks: (tile_lo, tile_hi).  Chunk 0 runs early (psum buffer 0 frees
# after fi8(14)); chunk 1 after fi8(15); chunk 2 (on buffer 0 again) after
# tile 15's interp.  Matmul psum dests never cross a 512-col bank boundary
# (m1 starts at the next bank).
MLP_CH = [(0, 8), (8, 12), (12, 16)]

_CACHE = {}


def build():
    nc = bacc.Bacc("TRN2", target_bir_lowering=False)

    aug1h = nc.dram_tensor("aug1h", [K, NLOC], bf16, kind="ExternalInput")
    aug2h = nc.dram_tensor("aug2h", [K, S], bf16, kind="ExternalInput")
    p1h = nc.dram_tensor("p1h", [D1, NLOC], bf16, kind="ExternalInput")
    p2bf = nc.dram_tensor("p2bf", [S, D2], bf16, kind="ExternalInput")
    w0h = nc.dram_tensor("w0h", [128, 3, M0], bf16, kind="ExternalInput")
    w1h = nc.dram_tensor("w1h", [128, 2, M1], bf16, kind="ExternalInput")
    bnh = nc.dram_tensor("bnh", [128, 3], f32, kind="ExternalInput")
    out = nc.dram_tensor("out", [M1, NLOC], f32, kind="ExternalOutput")

    AL = mybir.AluOpType
    ACT = mybir.ActivationFunctionType

    with TileContext(nc) as tc:
        with tc.tile_pool(name="const", bufs=1) as cp:
            a1 = cp.tile([K, NLOC], bf16)
            a2 = cp.tile([K, S], bf16)
            p1 = cp.tile([D1, NLOC], bf16)
            w0 = cp.tile([128, 3, M0], bf16)
            w1 = cp.tile([128, 2, M1], bf16)
            bn = cp.tile([128, 3], f32)
            identf = cp.tile([128, 128], f32)
            interp_all = cp.tile([128, 2, NLOC], bf16)
            h0s = [cp.tile([128, 2, 128 * (hi - lo)], bf16, name=f"h0_{ci}")
                   for ci, (lo, hi) in enumerate(MLP_CH)]
            osbs = [cp.tile([128, 128 * (hi - lo)], f32, name=f"osb_{ci}")
                    for ci, (lo, hi) in enumerate(MLP_CH)]

            # aug tiles gate the whole pipeline: load them first, on two
            # parallel queues.
            nc.sync.dma_start(a1[:], aug1h[:])
            nc.scalar.dma_start(a2[:], aug2h[:])
            nc.scalar.dma_start(p1[:], p1h[:])
            nc.scalar.dma_start(w0[:], w0h[:])
            nc.scalar.dma_start(w1[:], w1h[:])
            nc.scalar.dma_start(bn[:], bnh[:])
            make_identity(nc, identf[:])

            with tc.tile_pool(name="kps", bufs=2, space="PSUM") as kps, \
                 tc.tile_pool(name="selp", bufs=4) as selp, \
                 tc.tile_pool(name="mxp", bufs=1) as mxp, \
                 tc.tile_pool(name="wp", bufs=4) as wp, \
                 tc.tile_pool(name="gp", bufs=3) as gp, \
                 tc.tile_pool(name="ip", bufs=3) as ip:

                state = {}
                pks = {}

                def head(t):
                    pk = kps.tile([128, S], f32, tag="pk")
                    # Bank t%3 holds the transpose region that tail_a(t-3)
                    # carved (copy-read releases it last): emit it LAST so
                    # the other three dist matmuls start without waiting.
                    rb = t % 3
                    for b in [3] + [b for b in range(3) if b != rb] + [rb]:
                        nc.tensor.matmul(
                            pk[:, 512 * b:512 * (b + 1)],
                            lhsT=a1[:, 128 * t:128 * (t + 1)],
                            rhs=a2[:, 512 * b:512 * (b + 1)],
                            start=True, stop=True)
                    mx = mxp.tile([128, 8], f32, tag="mx")
                    ix = selp.tile([128, 8], u32, tag="ix")
                    nc.vector.max(out=mx[:], in_=pk[:])
                    # d6 = [mx0..2, mx0..2] on DVE right after max8: keeps
                    # every mx reader on the DVE queue so mx's bufs=1 WAR
                    # never crosses engines (max8(t+1) follows DVE program
                    # order).  No negation/epsilon needed: the pair products
                    # pw_k = mx_j*mx_l cancel the sign, and exact-zero
                    # distances still give the right one-hot weights.
                    d6 = wp.tile([128, 2, 3], f32, tag="d6")
                    mx13 = mx[:, 0:3].rearrange("p (o a) -> p o a", o=1)
                    nc.vector.tensor_scalar(
                        out=d6[:], in0=mx13.broadcast_to([128, 2, 3]),
                        scalar1=0.0, scalar2=None, op0=AL.add)
                    # previous tile's 1/sum slots between max8 and MVL/FI8.
                    if t - 1 in state:
                        st = state[t - 1]
                        nc.vector.reciprocal(st["sv"][:], st["sm"][:, 1:2])
                    nc.vector.max_index(out=ix[:], in_max=mx[:], in_values=pk[:])
                    state[t] = {"mx": mx, "ix": ix, "d6": d6}
                    pks[t] = pk

                def head_rest(t):
                    st = state[t]
                    ix, d6 = st["ix"], st["d6"]
                    d6f = d6.rearrange("p a b -> p (a b)")
                    # weights: w_k = prod_{j!=k} mx_j / sum_k(prod) -- the
                    # whole chain (products, sum, divide) runs on gpsimd.
                    pw = wp.tile([128, 3], f32, tag="pw")
                    sm = wp.tile([128, 2], f32, tag="sm")
                    sv = wp.tile([128, 1], f32, tag="sv")
                    w3 = wp.tile([128, 3], f32, tag="w3")
                    nc.gpsimd.tensor_tensor(out=pw[:], in0=d6f[:, 1:4],
                                            in1=d6f[:, 2:5], op=AL.mult)
                    nc.gpsimd.tensor_tensor(out=sm[:, 0:1], in0=pw[:, 0:1],
                                            in1=pw[:, 1:2], op=AL.add)
                    nc.gpsimd.tensor_tensor(out=sm[:, 1:2], in0=sm[:, 0:1],
                                            in1=pw[:, 2:3], op=AL.add)


                    g = gp.tile([128, 3, D2], bf16, tag="g")
                    if GATHER_BATCHED:
                        # one batched indirect DMA for all 3 neighbor rows,
                        # from a densely packed offset table (DVE bit-copy)
                        ixd = wp.tile([128, 4], u32, tag="ixd")
                        nc.vector.tensor_scalar(
                            out=ixd[:, 0:3], in0=ix[:, 0:3], scalar1=0,
                            scalar2=None, op0=AL.bitwise_or)
                        nc.gpsimd.indirect_dma_start(
                            out=g[:, :, :], out_offset=None, in_=p2bf[:],
                            in_offset=bass.IndirectOffsetOnAxis(
                                ap=ixd[:, 0:3], axis=0))
                    else:
                        for k in range(3):
                            nc.gpsimd.indirect_dma_start(
                                out=g[:, k, :], out_offset=None, in_=p2bf[:],
                                in_offset=bass.IndirectOffsetOnAxis(
                                    ap=ix[:, k:k + 1], axis=0))
                    st.update(g=g, pw=pw, sm=sm, sv=sv, w3=w3)

                def tail_dg(t):
                    # dg_k = diag(w_k): built on ACT in parallel with the
                    # gather (only needs w3, not g).
                    st = state[t]
                    nc.scalar.activation(out=st["w3"][:], in_=st["pw"][:],
                                         func=ACT.Copy, bias=0.0,
                                         scale=st["sv"][:])
                    dg = ip.tile([128, 3, 128], bf16, tag="dg")
                    for k in range(3):
                        nc.scalar.activation(out=dg[:, k, :], in_=identf[:],
                                             func=ACT.Copy, bias=0.0,
                                             scale=st["w3"][:, k:k + 1])
                    st["dg"] = dg

                def hostbuf(u):
                    # psum buffer that frees up at iteration u (same physical
                    # buffer parity once past the last tile).
                    while u >= NT:
                        u -= 2
                    return pks[u]

                def tail_a(t, buf, bases):
                    # weighted sum + transpose-to-feature-major via the
                    # diag(w) matmuls, into two 128-col psum slots of `buf`
                    # (rotating so consecutive tiles never WAR-serialize),
                    # ACT-copied out.
                    st = state[t]
                    g, dg = st["g"], st["dg"]
                    for h in range(2):
                        psI = buf[:, bases[h]:bases[h] + 128]
                        for k in range(3):
                            nc.tensor.matmul(
                                psI,
                                lhsT=g[:, k, 128 * h:128 * (h + 1)],
                                rhs=dg[:, k, :],
                                start=(k == 0), stop=(k == 2))
                        nc.scalar.activation(
                            out=interp_all[:, h, 128 * t:128 * (t + 1)],
                            in_=buf[:, bases[h]:bases[h] + 128],
                            func=ACT.Copy)

                def mlp_l0(ci, buf):
                    # layer 0: m0 at column 0, m1 at the next bank boundary
                    # so no matmul dest crosses a 512-col psum bank.
                    t_lo, t_hi = MLP_CH[ci]
                    cols0 = 128 * t_lo
                    w = 128 * (t_hi - t_lo)
                    wb = (w + 511) // 512 * 512
                    for m in range(2):
                        for cc in range(0, w, 512):
                            cw = min(512, w - cc)
                            ps = buf[:, wb * m + cc:wb * m + cc + cw]
                            for ki in range(3):
                                rhs = (p1[:, cols0 + cc:cols0 + cc + cw]
                                       if ki == 0 else
                                       interp_all[:, ki - 1,
                                                  cols0 + cc:cols0 + cc + cw])
                                nc.tensor.matmul(
                                    ps,
                                    lhsT=w0[:, ki, 128 * m:128 * (m + 1)],
                                    rhs=rhs,
                                    start=(ki == 0), stop=(ki == 2))

                def mlp_rest(ci, buf):
                    t_lo, t_hi = MLP_CH[ci]
                    cols0 = 128 * t_lo
                    w = 128 * (t_hi - t_lo)
                    wb = (w + 511) // 512 * 512
                    h0 = h0s[ci]
                    for m in range(2):
                        nc.scalar.activation(out=h0[:, m, :],
                                             in_=buf[:, wb * m:wb * m + w],
                                             func=ACT.Relu,
                                             bias=bn[:, m:m + 1], scale=1.0)
                    # layer 1 into buf[:, 0:w] (WAR on relu reads is tracked)
                    for cc in range(0, w, 512):
                        cw = min(512, w - cc)
                        ps1 = buf[:, cc:cc + cw]
                        for ki in range(2):
                            nc.tensor.matmul(
                                ps1, lhsT=w1[:, ki, :],
                                rhs=h0[:, ki, cc:cc + cw],
                                start=(ki == 0), stop=(ki == 1))
                    osb = osbs[ci]
                    nc.scalar.activation(out=osb[:], in_=buf[:, 0:w],
                                         func=ACT.Relu, bias=bn[:, 2:3],
                                         scale=1.0)
                    nc.sync.dma_start(out[:, cols0:cols0 + w], osb[:])

                # Emission order per iteration shapes each in-order engine
                # queue (as in the tuned 148us kernel): PE [dist(u),
                # transpose(u-2)], ACT [interp-copy(u-2), w3(u-1), dg(u-1),
                # d6(u)], gpsimd [TT(u), gathers(u)], DVE [max8(u),
                # recip(u-1), MVL(u), fi8(u)].
                for u in range(NT):
                    head(u)
                    if u >= 2:
                        b = 640 * ((u - 2) % 3)
                        tail_a(u - 2, hostbuf(u - 1), [b, b + 128])
                    if u >= 1:
                        tail_dg(u - 1)
                    head_rest(u)
                    if u == NT - 1:
                        # psum buffer 0 frees at fi8(14): start the MLP on
                        # tiles 0-7 while tile 15's selection runs.
                        mlp_l0(0, pks[NT - 2])

                st15 = state[NT - 1]
                nc.vector.reciprocal(st15["sv"][:], st15["sm"][:, 1:2])
                tail_dg(NT - 1)
                mlp_rest(0, pks[NT - 2])
                # the last two transposes go into psum buffer 1 regions the
                # MLP never touches there ([1280:1536] and [1792:2048]).
                tail_a(NT - 2, pks[NT - 1], [1280, 1408])
                mlp_l0(1, pks[NT - 1])
                tail_a(NT - 1, pks[NT - 1], [1792, 1920])
                mlp_rest(1, pks[NT - 1])
                mlp_l0(2, pks[NT - 2])
                mlp_rest(2, pks[NT - 2])

    nc.finalize()
    return nc


def _split3(x):
    """Lossless-ish 3xbf16 decomposition: x ~= h + m + l."""
    h = x.astype(ml_dtypes.bfloat16)
    r = x - h.astype(np.float32)
    m = r.astype(ml_dtypes.bfloat16)
    l = (r - m.astype(np.float32)).astype(ml_dtypes.bfloat16)
    return h, m, l


def _aug_pair(x1, x2):
    """Build the K=24 bf16 row stacks for psum = -|x1-x2|^2."""
    n = x1.shape[1]
    s = x2.shape[1]
    t2 = 2.0 * x2
    h1, m1, l1 = _split3(x1)
    h2, m2, l2 = _split3(t2)
    s1 = np.sum(x1 * x1, axis=0, dtype=np.float64).astype(np.float32)
    s2 = np.sum(x2 * x2, axis=0, dtype=np.float64).astype(np.float32)
    s1h, s1m, s1l = _split3(s1[None, :])
    s2h, s2m, s2l = _split3(s2[None, :])

    neg3 = np.full((3, 1), -1.0, ml_dtypes.bfloat16)
    aug1 = np.concatenate([
        h1, h1, m1, m1, h1, l1,                       # rows 0-17 (lhs splits)
        np.repeat(neg3, n, axis=1),                   # rows 18-20: -1
        np.concatenate([s1h, s1m, s1l], axis=0),      # rows 21-23: |x1|^2 hml
    ], axis=0).astype(ml_dtypes.bfloat16)
    aug2 = np.concatenate([
        h2, m2, h2, m2, l2, h2,                       # rows 0-17 (rhs splits)
        np.concatenate([s2h, s2m, s2l], axis=0),      # rows 18-20: |x2|^2 hml
        np.repeat(neg3, s, axis=1),                   # rows 21-23: -1
    ], axis=0).astype(ml_dtypes.bfloat16)
    return np.ascontiguousarray(aug1), np.ascontiguousarray(aug2)


def make_in_maps(inputs):
    xyz1 = np.asarray(inputs["xyz1"], np.float32)
    xyz2 = np.asarray(inputs["xyz2"], np.float32)
    points1 = np.asarray(inputs["points1"], np.float32)
    points2 = np.asarray(inputs["points2"], np.float32)

    p2bf = np.ascontiguousarray(points2.T).astype(ml_dtypes.bfloat16)

    # BN (inference) folded into the conv weights/bias:
    #   y = relu(x@(W*s) + c),  s = g/sqrt(v+eps),  c = (b-m)*s + be
    def fold(Wk, bk, gk, bek, mk, vk):
        W = np.asarray(inputs[Wk], np.float64)
        b = np.asarray(inputs[bk], np.float64)
        g = np.asarray(inputs[gk], np.float64)
        be = np.asarray(inputs[bek], np.float64)
        m = np.asarray(inputs[mk], np.float64)
        v = np.asarray(inputs[vk], np.float64)
        s = g / np.sqrt(v + BN_EPS)
        return (W * s[None, :]).astype(np.float32), ((b - m) * s + be).astype(np.float32)

    W0f, c0 = fold("W0", "b0", "g0", "be0", "m0", "v0")
    W1f, c1 = fold("W1", "b1", "g1", "be1", "m1", "v1")
    w0h = np.ascontiguousarray(
        W0f.reshape(3, 128, M0).transpose(1, 0, 2)).astype(ml_dtypes.bfloat16)
    w1h = np.ascontiguousarray(
        W1f.reshape(2, 128, M1).transpose(1, 0, 2)).astype(ml_dtypes.bfloat16)
    bnh = np.ascontiguousarray(
        np.stack([c0[0:128], c0[128:256], c1], axis=1)).astype(np.float32)

    in_maps = []
    for c in range(NCORES):
        sl = slice(c * NLOC, (c + 1) * NLOC)
        aug1, aug2 = _aug_pair(xyz1[:, sl], xyz2)
        in_maps.append(dict(
            aug1h=aug1, aug2h=aug2,
            p1h=np.ascontiguousarray(points1[:, sl]).astype(ml_dtypes.bfloat16),
            p2bf=p2bf, w0h=w0h, w1h=w1h, bnh=bnh,
        ))
    return in_maps


def run(inputs, trace=False, **kwargs):
    if "nc" not in _CACHE:
        _CACHE["nc"] = build()
    nc = _CACHE["nc"]
    in_maps = make_in_maps(inputs)
    res = bass_utils.run_bass_kernel_spmd(
        nc, in_maps, core_ids=list(range(NCORES)), trace=trace, **kwargs)
    outs = [res.results[c]["out"] for c in range(NCORES)]
    full = np.concatenate(outs, axis=1)
    return full, res


def kernel(**inputs):
    full, _ = run(inputs, trace=False)
    return full


# revision 36
# speedup vs baseline: 1.0347x; 1.0347x over previous
"""PointNet Feature Propagation kernel for Trainium2 (8 NeuronCores, SPMD).

Data-parallel over N: each core owns 2048 of 16384 query points; xyz2/points2
and the conv/BN params are replicated.

Pipeline per core, streamed tile-by-tile (16 tiles of 128 points):
  1. Distance matrix as ONE K=24 bf16 matmul group per tile: CPU pre-encodes
     xyz as hi/mid/lo bf16 triples and stacks the cross terms PLUS the
     -|x1|^2 / -|x2|^2 norm rows along K, so psum = -d^2 exactly to ~1e-7.
     The dist matmul whose bank holds the rotating transpose region is
     emitted LAST so the other three never wait on the interp-copy release.
  2. DVE is the roofline engine and sets the ~5.5us/tile cadence: max8 +
     find_index8 (the two unavoidable full-S passes) plus tiny d6 ops and
     the weight reciprocal.  d6 is computed ON the DVE right after max8 so
     every reader of the bufs=1 mx tile is a DVE op: max8(t+1)'s WAR on mx
     then follows pure DVE program order, which stops the Tile scheduler
     from statically hoisting max8(t+1) ahead of fi8(t) (that pairing
     couples one tile's gather->transpose->copy chain serially with two
     full DVE scans and was worth ~25us).
  3. Neighbor rows: 3 per-k indirect row-gathers per tile from a bf16 DRAM
     table (HW SWDGE only honors one offset per partition - batched
     multi-offset gathers return garbage on HW even though CoreSim accepts
     them).
  4. dg_k = diag(w_k) built on ACT concurrently with the gathers; the PE
     then does weighted-sum + transpose-to-feature-major in one 3-matmul
     accumulation group per 128-col half, into rotating 2x128-col psum
     slots carved from the opposite-parity distance buffer, ACT-copied out.
  5. The two 1x1-conv layers (BN folded on CPU, ReLU+bias fused into one
     ACT per chunk, all psum dests bank-aligned) run as three chunks:
     tiles 0-7 start right after fi8(14) frees psum buffer 0 (overlapping
     tile 15's selection); tiles 8-11 and 12-15 follow.
"""
import ml_dtypes
import numpy as np

import concourse.bacc as bacc
import concourse.bass as bass
import concourse.mybir as mybir
from concourse import bass_utils
from concourse.masks import make_identity
from concourse.tile import TileContext

f32 = mybir.dt.float32
bf16 = mybir.dt.bfloat16
u32 = mybir.dt.uint32

NCORES = 8
N = 16384
NLOC = N // NCORES          # 2048 points per core
S = 2048                    # reference points (replicated)
D1 = 128                    # points1 channels
D2 = 256                    # points2 channels
M0 = 256                    # mlp hidden
M1 = 128                    # mlp out
NT = NLOC // 128            # 16 point-tiles per core
K = 24                      # augmented contraction rows
BN_EPS = 1e-5
GATHER_BATCHED = False      # HW SWDGE only supports 1 offset/partition

# MLP chunks: (tile_lo, tile_hi).  Chunk 0 runs early (psum buffer 0 frees
# after fi8(14)); chunk 1 after fi8(15); chunk 2 (on buffer 0 again) after
# tile 15's interp.  Matmul psum dests never cross a 512-col bank boundary
# (m1 starts at the next bank).
MLP_CH = [(0, 8), (8, 12), (12, 16)]

_CACHE = {}


def build():
    nc = bacc.Bacc("TRN2", target_bir_lowering=False)

    aug1h = nc.dram_tensor("aug1h", [K, NLOC], bf16, kind="ExternalInput")
    aug2h = nc.dram_tensor("aug2h", [K, S], bf16, kind="ExternalInput")
    p1h = nc.dram_tensor("p1h", [D1, NLOC], bf16, kind="ExternalInput")
    p2bf = nc.dram_tensor("p2bf", [S, D2], bf16, kind="ExternalInput")
    w0h = nc.dram_tensor("w0h", [128, 3, M0], bf16, kind="ExternalInput")
    w1h = nc.dram_tensor("w1h", [128, 2, M1], bf16, kind="ExternalInput")
    bnh = nc.dram_tensor("bnh", [128, 3], f32, kind="ExternalInput")
    out = nc.dram_tensor("out", [M1, NLOC], f32, kind="ExternalOutput")

    AL = mybir.AluOpType
    ACT = mybir.ActivationFunctionType

    with TileContext(nc) as tc:
        with tc.tile_pool(name="const", bufs=1) as cp:
            a1 = cp.tile([K, NLOC], bf16)
            a2 = cp.tile([K, S], bf16)
            p1 = cp.tile([D1, NLOC], bf16)
            w0 = cp.tile([128, 3, M0], bf16)
            w1 = cp.tile([128, 2, M1], bf16)
            bn = cp.tile([128, 3], f32)
            identf = cp.tile([128, 128], f32)
            interp_all = cp.tile([128, 2, NLOC], bf16)
            h0s = [cp.tile([128, 2, 128 * (hi - lo)], bf16, name=f"h0_{ci}")
                   for ci, (lo, hi) in enumerate(MLP_CH)]
            osbs = [cp.tile([128, 128 * (hi - lo)], f32, name=f"osb_{ci}")
                    for ci, (lo, hi) in enumerate(MLP_CH)]

            # aug tiles gate the whole pipeline: load them first, on two
            # parallel queues, chunked so tile 0's operands (a1 cols 0:128,
            # a2 in its bank emission order [3,1,2,0]) land first and the
            # first distance matmuls start as early as possible.
            nc.sync.dma_start(a1[:, 0:256], aug1h[:, 0:256])
            nc.sync.dma_start(a1[:, 256:NLOC], aug1h[:, 256:NLOC])
            for b in (3, 1, 2, 0):
                nc.scalar.dma_start(a2[:, 512 * b:512 * (b + 1)],
                                    aug2h[:, 512 * b:512 * (b + 1)])
            nc.scalar.dma_start(p1[:], p1h[:])
            nc.scalar.dma_start(w0[:], w0h[:])
            nc.scalar.dma_start(w1[:], w1h[:])
            nc.scalar.dma_start(bn[:], bnh[:])
            make_identity(nc, identf[:])

            with tc.tile_pool(name="kps", bufs=2, space="PSUM") as kps, \
                 tc.tile_pool(name="selp", bufs=4) as selp, \
                 tc.tile_pool(name="mxp", bufs=1) as mxp, \
                 tc.tile_pool(name="wp", bufs=4) as wp, \
                 tc.tile_pool(name="gp", bufs=3) as gp, \
                 tc.tile_pool(name="ip", bufs=3) as ip:

                state = {}
                pks = {}

                def head(t):
                    pk = kps.tile([128, S], f32, tag="pk")
                    # Bank t%3 holds the transpose region that tail_a(t-3)
                    # carved (copy-read releases it last): emit it LAST so
                    # the other three dist matmuls start without waiting.
                    rb = t % 3
                    for b in [3] + [b for b in range(3) if b != rb] + [rb]:
                        nc.tensor.matmul(
                            pk[:, 512 * b:512 * (b + 1)],
                            lhsT=a1[:, 128 * t:128 * (t + 1)],
                            rhs=a2[:, 512 * b:512 * (b + 1)],
                            start=True, stop=True)
                    mx = mxp.tile([128, 8], f32, tag="mx")
                    ix = selp.tile([128, 8], u32, tag="ix")
                    nc.vector.max(out=mx[:], in_=pk[:])
                    # d6 on DVE right after max8: keeps every mx reader on
                    # the DVE queue so mx's bufs=1 WAR never crosses engines
                    # (max8(t+1) is gated purely by DVE program order).
                    d6 = wp.tile([128, 6], f32, tag="d6")
                    nc.vector.tensor_scalar(out=d6[:, 0:3], in0=mx[:, 0:3],
                                            scalar1=-1.0, scalar2=1e-8,
                                            op0=AL.mult, op1=AL.add)
                    nc.vector.tensor_scalar(out=d6[:, 3:6], in0=mx[:, 0:3],
                                            scalar1=-1.0, scalar2=1e-8,
                                            op0=AL.mult, op1=AL.add)
                    # previous tile's 1/sum slots between max8 and MVL/FI8.
                    if t - 1 in state:
                        st = state[t - 1]
                        nc.vector.reciprocal(st["sv"][:], st["sm"][:, 1:2])
                    nc.vector.max_index(out=ix[:], in_max=mx[:], in_values=pk[:])
                    state[t] = {"mx": mx, "ix": ix, "d6": d6}
                    pks[t] = pk

                def head_rest(t):
                    st = state[t]
                    ix, d6 = st["ix"], st["d6"]
                    # weights: d'_k = d^2_k + 1e-8 = -mx_k + 1e-8;
                    # w_k = prod_{j!=k} d'_j / sum_k(prod) -- one reciprocal.
                    pw = wp.tile([128, 3], f32, tag="pw")
                    sm = wp.tile([128, 2], f32, tag="sm")
                    sv = wp.tile([128, 1], f32, tag="sv")
                    w3 = wp.tile([128, 3], f32, tag="w3")
                    nc.gpsimd.tensor_tensor(out=pw[:], in0=d6[:, 1:4],
                                            in1=d6[:, 2:5], op=AL.mult)
                    nc.gpsimd.tensor_tensor(out=sm[:, 0:1], in0=pw[:, 0:1],
                                            in1=pw[:, 1:2], op=AL.add)
                    nc.gpsimd.tensor_tensor(out=sm[:, 1:2], in0=sm[:, 0:1],
                                            in1=pw[:, 2:3], op=AL.add)

                    g = gp.tile([128, 3, D2], bf16, tag="g")
                    if GATHER_BATCHED:
                        # one batched indirect DMA for all 3 neighbor rows,
                        # from a densely packed offset table (DVE bit-copy)
                        ixd = wp.tile([128, 4], u32, tag="ixd")
                        nc.vector.tensor_scalar(
                            out=ixd[:, 0:3], in0=ix[:, 0:3], scalar1=0,
                            scalar2=None, op0=AL.bitwise_or)
                        nc.gpsimd.indirect_dma_start(
                            out=g[:, :, :], out_offset=None, in_=p2bf[:],
                            in_offset=bass.IndirectOffsetOnAxis(
                                ap=ixd[:, 0:3], axis=0))
                    else:
                        for k in range(3):
                            nc.gpsimd.indirect_dma_start(
                                out=g[:, k, :], out_offset=None, in_=p2bf[:],
                                in_offset=bass.IndirectOffsetOnAxis(
                                    ap=ix[:, k:k + 1], axis=0))
                    st.update(g=g, pw=pw, sm=sm, sv=sv, w3=w3)

                def tail_dg(t):
                    # dg_k = diag(w_k): built on ACT in parallel with the
                    # gather (only needs w3, not g).
                    st = state[t]
                    nc.scalar.activation(out=st["w3"][:], in_=st["pw"][:],
                                         func=ACT.Copy, bias=0.0,
                                         scale=st["sv"][:])
                    dg = ip.tile([128, 3, 128], bf16, tag="dg")
                    for k in range(3):
                        nc.scalar.activation(out=dg[:, k, :], in_=identf[:],
                                             func=ACT.Copy, bias=0.0,
                                             scale=st["w3"][:, k:k + 1])
                    st["dg"] = dg

                def hostbuf(u):
                    # psum buffer that frees up at iteration u (same physical
                    # buffer parity once past the last tile).
                    while u >= NT:
                        u -= 2
                    return pks[u]

                def tail_a(t, buf, bases):
                    # weighted sum + transpose-to-feature-major via the
                    # diag(w) matmuls, into two 128-col psum slots of `buf`
                    # (rotating so consecutive tiles never WAR-serialize),
                    # ACT-copied out.
                    st = state[t]
                    g, dg = st["g"], st["dg"]
                    for h in range(2):
                        psI = buf[:, bases[h]:bases[h] + 128]
                        for k in range(3):
                            nc.tensor.matmul(
                                psI,
                                lhsT=g[:, k, 128 * h:128 * (h + 1)],
                                rhs=dg[:, k, :],
                                start=(k == 0), stop=(k == 2))
                        nc.scalar.activation(
                            out=interp_all[:, h, 128 * t:128 * (t + 1)],
                            in_=buf[:, bases[h]:bases[h] + 128],
                            func=ACT.Copy)

                def mlp_l0(ci, buf):
                    # layer 0: m0 at column 0, m1 at the next bank boundary
                    # so no matmul dest crosses a 512-col psum bank.
                    t_lo, t_hi = MLP_CH[ci]
                    cols0 = 128 * t_lo
                    w = 128 * (t_hi - t_lo)
                    wb = (w + 511) // 512 * 512
                    for m in range(2):
                        for cc in range(0, w, 512):
                            cw = min(512, w - cc)
                            ps = buf[:, wb * m + cc:wb * m + cc + cw]
                            for ki in range(3):
                                rhs = (p1[:, cols0 + cc:cols0 + cc + cw]
                                       if ki == 0 else
                                       interp_all[:, ki - 1,
                                                  cols0 + cc:cols0 + cc + cw])
                                nc.tensor.matmul(
                                    ps,
                                    lhsT=w0[:, ki, 128 * m:128 * (m + 1)],
                                    rhs=rhs,
                                    start=(ki == 0), stop=(ki == 2))

                def mlp_rest(ci, buf):
                    t_lo, t_hi = MLP_CH[ci]
                    cols0 = 128 * t_lo
                    w = 128 * (t_hi - t_lo)
                    wb = (w + 511) // 512 * 512
                    h0 = h0s[ci]
                    for m in range(2):
                        nc.scalar.activation(out=h0[:, m, :],
                                             in_=buf[:, wb * m:wb * m + w],
                                             func=ACT.Relu,
                                             bias=bn[:, m:m + 1], scale=1.0)
                    # layer 1 into buf[:, 0:w] (WAR on relu reads is tracked)
                    for cc in range(0, w, 512):
                        cw = min(512, w - cc)
                        ps1 = buf[:, cc:cc + cw]
                        for ki in range(2):
                            nc.tensor.matmul(
                                ps1, lhsT=w1[:, ki, :],
                                rhs=h0[:, ki, cc:cc + cw],
                                start=(ki == 0), stop=(ki == 1))
                    osb = osbs[ci]
                    nc.scalar.activation(out=osb[:], in_=buf[:, 0:w],
                                         func=ACT.Relu, bias=bn[:, 2:3],
                                         scale=1.0)
                    nc.sync.dma_start(out[:, cols0:cols0 + w], osb[:])

                # Emission order per iteration shapes each in-order engine
                # queue (as in the tuned 148us kernel): PE [dist(u),
                # transpose(u-2)], ACT [interp-copy(u-2), w3(u-1), dg(u-1),
                # d6(u)], gpsimd [TT(u), gathers(u)], DVE [max8(u),
                # recip(u-1), MVL(u), fi8(u)].
                for u in range(NT):
                    head(u)
                    if u >= 2:
                        b = 640 * ((u - 2) % 3)
                        tail_a(u - 2, hostbuf(u - 1), [b, b + 128])
                    if u >= 1:
                        tail_dg(u - 1)
                    head_rest(u)
                    if u == NT - 1:
                        # psum buffer 0 frees at fi8(14): start the MLP on
                        # tiles 0-7 while tile 15's selection runs.
                        mlp_l0(0, pks[NT - 2])

                st15 = state[NT - 1]
                nc.vector.reciprocal(st15["sv"][:], st15["sm"][:, 1:2])
                tail_dg(NT - 1)
                mlp_rest(0, pks[NT - 2])
                # the last two transposes go into psum buffer 1 regions the
                # MLP never touches there ([1280:1536] and [1792:2048]).
                mlp_l0(1, pks[NT - 1])
                tail_a(NT - 2, pks[NT - 1], [1280, 1408])
                tail_a(NT - 1, pks[NT - 1], [1792, 1920])
                mlp_rest(1, pks[NT - 1])
                mlp_l0(2, pks[NT - 2])
                mlp_rest(2, pks[NT - 2])

    nc.finalize()
    return nc


def _split3(x):
    """Lossless-ish 3xbf16 decomposition: x ~= h + m + l."""
    h = x.astype(ml_dtypes.bfloat16)
    r = x - h.astype(np.float32)
    m = r.astype(ml_dtypes.bfloat16)
    l = (r - m.astype(np.float32)).astype(ml_dtypes.bfloat16)
    return h, m, l


def _aug_pair(x1, x2):
    """Build the K=24 bf16 row stacks for psum = -|x1-x2|^2."""
    n = x1.shape[1]
    s = x2.shape[1]
    t2 = 2.0 * x2
    h1, m1, l1 = _split3(x1)
    h2, m2, l2 = _split3(t2)
    s1 = np.sum(x1 * x1, axis=0, dtype=np.float64).astype(np.float32)
    s2 = np.sum(x2 * x2, axis=0, dtype=np.float64).astype(np.float32)
    s1h, s1m, s1l = _split3(s1[None, :])
    s2h, s2m, s2l = _split3(s2[None, :])

    neg3 = np.full((3, 1), -1.0, ml_dtypes.bfloat16)
    aug1 = np.concatenate([
        h1, h1, m1, m1, h1, l1,                       # rows 0-17 (lhs splits)
        np.repeat(neg3, n, axis=1),                   # rows 18-20: -1
        np.concatenate([s1h, s1m, s1l], axis=0),      # rows 21-23: |x1|^2 hml
    ], axis=0).astype(ml_dtypes.bfloat16)
    aug2 = np.concatenate([
        h2, m2, h2, m2, l2, h2,                       # rows 0-17 (rhs splits)
        np.concatenate([s2h, s2m, s2l], axis=0),      # rows 18-20: |x2|^2 hml
        np.repeat(neg3, s, axis=1),                   # rows 21-23: -1
    ], axis=0).astype(ml_dtypes.bfloat16)
    return np.ascontiguousarray(aug1), np.ascontiguousarray(aug2)


def make_in_maps(inputs):
    xyz1 = np.asarray(inputs["xyz1"], np.float32)
    xyz2 = np.asarray(inputs["xyz2"], np.float32)
    points1 = np.asarray(inputs["points1"], np.float32)
    points2 = np.asarray(inputs["points2"], np.float32)

    p2bf = np.ascontiguousarray(points2.T).astype(ml_dtypes.bfloat16)

    # BN (inference) folded into the conv weights/bias:
    #   y = relu(x@(W*s) + c),  s = g/sqrt(v+eps),  c = (b-m)*s + be
    def fold(Wk, bk, gk, bek, mk, vk):
        W = np.asarray(inputs[Wk], np.float64)
        b = np.asarray(inputs[bk], np.float64)
        g = np.asarray(inputs[gk], np.float64)
        be = np.asarray(inputs[bek], np.float64)
        m = np.asarray(inputs[mk], np.float64)
        v = np.asarray(inputs[vk], np.float64)
        s = g / np.sqrt(v + BN_EPS)
        return (W * s[None, :]).astype(np.float32), ((b - m) * s + be).astype(np.float32)

    W0f, c0 = fold("W0", "b0", "g0", "be0", "m0", "v0")
    W1f, c1 = fold("W1", "b1", "g1", "be1", "m1", "v1")
    w0h = np.ascontiguousarray(
        W0f.reshape(3, 128, M0).transpose(1, 0, 2)).astype(ml_dtypes.bfloat16)
    w1h = np.ascontiguousarray(
        W1f.reshape(2, 128, M1).transpose(1, 0, 2)).astype(ml_dtypes.bfloat16)
    bnh = np.ascontiguousarray(
        np.stack([c0[0:128], c0[128:256], c1], axis=1)).astype(np.float32)

    in_maps = []
    for c in range(NCORES):
        sl = slice(c * NLOC, (c + 1) * NLOC)
        aug1, aug2 = _aug_pair(xyz1[:, sl], xyz2)
        in_maps.append(dict(
            aug1h=aug1, aug2h=aug2,
            p1h=np.ascontiguousarray(points1[:, sl]).astype(ml_dtypes.bfloat16),
            p2bf=p2bf, w0h=w0h, w1h=w1h, bnh=bnh,
        ))
    return in_maps


def run(inputs, trace=False, **kwargs):
    if "nc" not in _CACHE:
        _CACHE["nc"] = build()
    nc = _CACHE["nc"]
    in_maps = make_in_maps(inputs)
    res = bass_utils.run_bass_kernel_spmd(
        nc, in_maps, core_ids=list(range(NCORES)), trace=trace, **kwargs)
    outs = [res.results[c]["out"] for c in range(NCORES)]
    full = np.concatenate(outs, axis=1)
    return full, res


def kernel(**inputs):
    full, _ = run(inputs, trace=False)
    return full
